# revision 31
# baseline (speedup 1.0000x reference)
"""Trainium2 Bass kernel for the DecoderSVM SNN decoder.

reference computation:
    curr[t,b,o] = einsum('bit,oi->tbo', inputs, W) + b         (I=182 -> O=2)
    syn_t = clip(alpha,0,1)*syn_{t-1} + curr_t                 (scan over T)
    mem_t = clip(beta,0,1)*mem_{t-1} + syn_t
    out = mem_rec transposed to [B, T, O]

Strategy (8 NeuronCores, batch-sharded 32 per core), fp8 DoubleRow:
  - x and the block-diagonal stationary weights are quantized to TRN fp8
    e4m3 on the host (offline-verified rel_err ~7.8e-3 vs the 2e-2 gate).
    This halves HBM traffic vs bf16 -- the kernel is DMA-bound.
  - x is extended host-side with a ones/16 row (bias, weights b*16) and
    a zero row, so I+2 = 184 rows = exactly 46 K=128 chunks and the
    whole GEMM is 23 uniform fp8 DoubleRow chunk pairs (2 K-tiles per
    matmul instruction = 2x PE rate; 216ns per 512-col pair measured).
  - Block-diagonal GEMM: K=128 partitions = 32 batches x 4 input rows;
    lhsT [128, 2, 64] holds W block-diagonally so matmuls emit PSUM
    [64=(b,o), T] directly in scan layout.
  - DMA: fp8 payloads travel as uint32 (bitcast back for the PE); each
    group is ONE full-128-partition dma_start, groups alternating
    between the two HW queues (partition-split dma_starts starve the
    per-partition DMA engines: ~230 vs ~420 GB/s measured).  Small
    first groups start the PE early; consts ride ahead on scalar.
  - Double recurrence: two chained VectorE tensor_tensor_scans per PSUM
    bank tile (~2.3ns/elem, dtype-independent).  syn/mem stored fp16;
    y returns fp16, host upcasts.  Scans chase the PSUM banks as the
    accumulation finishes; y tiles stream out on both queues.
"""

import numpy as np

B, I, T, O = 256, 182, 2000, 2
NCORES = 8
NB = B // NCORES          # 32 batches per core
ROWS = 4                  # input rows folded into K per chunk
NFULL = 45                # chunks of real input rows only
NCHUNK = 46               # 46 chunks x 4 rows = 184 = I + ones row + zero row
IEXT = 184                # x extended with row 182 = ones/16, row 183 = zeros
M = 2 * NB                # 64 = output partitions (b_local, o)
ONES_VAL = 2.0**-4        # exact in e4m3; bias rides in lhsT as b*16
TSPLIT = [512, 512, 512, 464]  # PSUM-bank-aligned time tiles
# DMA/layout groups: (first chunk, chunks in group).  Each group is ONE
# full-128-partition dma_start; groups alternate between the two HW DMA
# queues (measured: partition-split dma_starts starve the per-partition
# DMA engines -- 224 GB/s vs 400 GB/s for full-partition alternating).
# Small first groups let the PE start early; small last group keeps the
# post-stream matmul tail short.
GROUPS = [(0, 4), (4, 4), (8, 4), (12, 4), (16, 8), (24, 8), (32, 8), (40, 6)]

TRACE = False

_cache = {}


def chunk_rows(c):
    """Global input-row indices (length ROWS) covered by chunk c, matching
    the merged-group DMA layout: within a group of mg chunks starting at
    chunk c0 (row base r0 = 4*c0), partition (4b+i) holds contiguous rows
    r0 + mg*i + cc for cc in [0, mg)."""
    for c0, mg in GROUPS:
        if c0 <= c < c0 + mg:
            return [4 * c0 + mg * i + (c - c0) for i in range(ROWS)]
    raise ValueError(c)


def _ops():
    """Matmul op schedule: 23 uniform DoubleRow chunk pairs in arrival
    order (chunk 45 -- the bias/tail chunk -- arrives last)."""
    return [("pair", c) for c in range(0, NCHUNK, 2)]


def _build_nc():
    import concourse.bacc as bacc
    import concourse.bass as bass
    import concourse.mybir as mybir
    from concourse.tile import TileContext

    f32 = mybir.dt.float32
    f16 = mybir.dt.float16
    bf16 = mybir.dt.bfloat16
    fp8 = mybir.dt.float8e4
    u32 = mybir.dt.uint32

    nc = bacc.Bacc("TRN2", target_bir_lowering=False, debug=False)

    # fp8 payloads travel as uint32 (4 fp8 lanes per element): the DMA
    # engines are element-rate limited (~14 elem/ns), so 1-byte elements
    # cap a queue-fed engine at ~14 GB/s while 4-byte elements reach the
    # ~22.5 GB/s bus limit.  SBUF tiles are bitcast back to fp8 for the PE.
    x8 = nc.dram_tensor("x8", [NB, IEXT, T // 4], u32, kind="ExternalInput")
    lhsT_full = nc.dram_tensor(
        "lhsT_full", [128, NCHUNK * M // 4], u32, kind="ExternalInput"
    )
    alpha_bc = nc.dram_tensor("alpha_bc", [M, 512], f32, kind="ExternalInput")
    beta_bc = nc.dram_tensor("beta_bc", [M, 512], f16, kind="ExternalInput")
    y = nc.dram_tensor("y", [M, T], f16, kind="ExternalOutput")

    with TileContext(nc) as tc:
        with (
            tc.tile_pool(name="consts", bufs=1) as cpool,
            tc.tile_pool(name="xs", bufs=1) as xpool,
            tc.tile_pool(name="mems", bufs=1) as mpool,
            tc.tile_pool(name="psum", bufs=1, space=bass.MemorySpace.PSUM) as ppool,
        ):
            # consts ride ahead on the scalar queue; x leads on sync
            lwu = cpool.tile([128, NCHUNK, M // 4], u32)
            nc.scalar.dma_start(out=lwu[:], in_=lhsT_full[:])
            ab = cpool.tile([M, 512], f32)
            nc.scalar.dma_start(out=ab[:], in_=alpha_bc[:])
            bb = cpool.tile([M, 512], f16)
            nc.scalar.dma_start(out=bb[:], in_=beta_bc[:])

            # full x shard stays resident (92KB/partition in fp8)
            xbu = xpool.tile([128, NCHUNK, T // 4], u32)

            for gi, (c0, mg) in enumerate(GROUPS):
                r0 = 4 * c0
                nrow = ROWS * mg
                src = x8[:, r0 : r0 + nrow, :].rearrange(
                    "b (i cc) t -> b i cc t", i=ROWS, cc=mg
                )
                dst = xbu[:, c0 : c0 + mg, :]
                eng = nc.sync if gi % 2 == 0 else nc.scalar
                eng.dma_start(out=dst, in_=src)

            # fp8 views of the u32-packed tiles for the PE
            lw = lwu[:].bitcast(fp8)  # [128, NCHUNK, M]
            xb = xbu[:].bitcast(fp8)  # [128, NCHUNK, T]

            pt = ppool.tile([M, 2048], f32)

            ops = _ops()
            for k, (kind, c) in enumerate(ops):
                start = k == 0
                stop = k == len(ops) - 1
                off = 0
                for w in TSPLIT:
                    sl = slice(off, off + w)
                    nc.tensor.matmul(
                        pt[:, sl],
                        lw[:, c : c + 2, :],
                        xb[:, c : c + 2, sl],
                        start=start,
                        stop=stop,
                        perf_mode=mybir.MatmulPerfMode.DoubleRow,
                    )
                    off += w

            syn = mpool.tile([M, T], f16)
            mem = mpool.tile([M, T], f16)
            off = 0
            for ti, w in enumerate(TSPLIT):
                sl = slice(off, off + w)
                nc.vector.tensor_tensor_scan(
                    syn[:, sl],
                    ab[:, :w],
                    pt[:, sl],
                    initial=(0.0 if ti == 0 else syn[:, off - 1 : off]),
                    op0=mybir.AluOpType.mult,
                    op1=mybir.AluOpType.add,
                )
                nc.vector.tensor_tensor_scan(
                    mem[:, sl],
                    bb[:, :w],
                    syn[:, sl],
                    initial=(0.0 if ti == 0 else mem[:, off - 1 : off]),
                    op0=mybir.AluOpType.mult,
                    op1=mybir.AluOpType.add,
                )
                eng = nc.sync if ti % 2 == 0 else nc.scalar
                eng.dma_start(out=y[:, sl], in_=mem[:, sl])
                off += w

    nc.compile()
    return nc


def _host_tensors(W, b, alpha, beta):
    import ml_dtypes

    e4 = ml_dtypes.float8_e4m3
    W = np.asarray(W, np.float32)
    bvec = np.asarray(b, np.float32)
    a_cl = np.clip(np.asarray(alpha, np.float32), 0.0, 1.0)
    bt_cl = np.clip(np.asarray(beta, np.float32), 0.0, 1.0)

    # extended weight columns: row 182 multiplies the ones/16 row (bias),
    # row 183 multiplies the zero row
    W_ext = np.zeros((O, IEXT), np.float32)
    W_ext[:, :I] = W
    W_ext[:, I] = bvec / ONES_VAL

    bidx = np.arange(NB)
    lhsT = np.zeros((128, NCHUNK, M), np.float32)
    for c in range(NCHUNK):
        rows = chunk_rows(c)
        for i in range(ROWS):
            for o in range(O):
                lhsT[ROWS * bidx + i, c, 2 * bidx + o] = W_ext[o, rows[i]]

    lhsT_full = np.ascontiguousarray(
        lhsT.reshape(128, NCHUNK * M).astype(e4)
    ).view(np.uint32)
    alpha_bc = np.ascontiguousarray(
        np.broadcast_to(np.tile(a_cl, NB)[:, None], (M, 512))
    ).astype(np.float32)
    beta_bc = np.ascontiguousarray(
        np.broadcast_to(np.tile(bt_cl, NB)[:, None], (M, 512))
    ).astype(np.float16)
    return lhsT_full, alpha_bc, beta_bc


def kernel(inputs, W, b, alpha, beta):
    import ml_dtypes
    from concourse.bass_utils import run_bass_kernel_spmd

    if "fp8" not in _cache:
        _cache["fp8"] = _build_nc()
    nc = _cache["fp8"]

    lhsT_full, alpha_bc, beta_bc = _host_tensors(W, b, alpha, beta)
    # x extended with the ones/16 row (182) and a zero row (183)
    x8f = np.zeros((B, IEXT, T), ml_dtypes.float8_e4m3)
    x8f[:, :I] = np.asarray(inputs, np.float32).astype(ml_dtypes.float8_e4m3)
    x8f[:, I] = ml_dtypes.float8_e4m3(ONES_VAL)
    x8 = np.ascontiguousarray(x8f).view(np.uint32)

    in_maps = []
    for c in range(NCORES):
        in_maps.append(
            {
                "x8": np.ascontiguousarray(x8[c * NB : (c + 1) * NB]),
                "lhsT_full": lhsT_full,
                "alpha_bc": alpha_bc,
                "beta_bc": beta_bc,
            }
        )

    res = run_bass_kernel_spmd(nc, in_maps, core_ids=list(range(NCORES)), trace=TRACE)
    kernel.last_exec_time_ns = res.exec_time_ns
    kernel.last_result = res
    out = np.empty((B, O, T), np.float32)
    for c in range(NCORES):
        out[c * NB : (c + 1) * NB] = (
            res.results[c]["y"].astype(np.float32).reshape(NB, O, T)
        )
    return np.ascontiguousarray(out.transpose(0, 2, 1))


kernel.last_exec_time_ns = None
kernel.last_result = None


# revision 33
# speedup vs baseline: 1.0469x; 1.0469x over previous
"""Trainium2 Bass kernel for the DecoderSVM SNN decoder.

reference computation:
    curr[t,b,o] = einsum('bit,oi->tbo', inputs, W) + b         (I=182 -> O=2)
    syn_t = clip(alpha,0,1)*syn_{t-1} + curr_t                 (scan over T)
    mem_t = clip(beta,0,1)*mem_{t-1} + syn_t
    out = mem_rec transposed to [B, T, O]

Strategy (8 NeuronCores, batch-sharded 32 per core), fp8 DoubleRow:
  - x and the block-diagonal stationary weights are quantized to TRN fp8
    e4m3 on the host (offline-verified rel_err ~7.8e-3 vs the 2e-2 gate).
    This halves HBM traffic vs bf16 -- the kernel is DMA-bound.
  - x is extended host-side with a ones/16 row (bias, weights b*16) and
    a zero row, so I+2 = 184 rows = exactly 46 K=128 chunks and the
    whole GEMM is 23 uniform fp8 DoubleRow chunk pairs (2 K-tiles per
    matmul instruction = 2x PE rate; 216ns per 512-col pair measured).
  - Block-diagonal GEMM: K=128 partitions = 32 batches x 4 input rows;
    lhsT [128, 2, 64] holds W block-diagonally so matmuls emit PSUM
    [64=(b,o), T] directly in scan layout.
  - DMA: fp8 payloads travel as uint32 (bitcast back for the PE); each
    group is ONE full-128-partition dma_start, groups alternating
    between the two HW queues (partition-split dma_starts starve the
    per-partition DMA engines: ~230 vs ~420 GB/s measured).  Small
    first groups start the PE early; consts ride ahead on scalar.
  - Double recurrence: two chained VectorE tensor_tensor_scans per PSUM
    bank tile (~2.3ns/elem, dtype-independent).  syn/mem stored fp16;
    y returns fp16, host upcasts.  Scans chase the PSUM banks as the
    accumulation finishes; y tiles stream out on both queues.
"""

import numpy as np

B, I, T, O = 256, 182, 2000, 2
NCORES = 8
NB = B // NCORES          # 32 batches per core
ROWS = 4                  # input rows folded into K per chunk
NFULL = 45                # chunks of real input rows only
NCHUNK = 46               # 46 chunks x 4 rows = 184 = I + ones row + zero row
IEXT = 184                # x extended with row 182 = ones/16, row 183 = zeros
M = 2 * NB                # 64 = output partitions (b_local, o)
ONES_VAL = 2.0**-4        # exact in e4m3; bias rides in lhsT as b*16
TSPLIT = [512, 512, 512, 464]  # PSUM-bank-aligned time tiles
# DMA/layout groups: (first chunk, chunks in group).  Each group is ONE
# full-128-partition dma_start; groups alternate between the two HW DMA
# queues (measured: partition-split dma_starts starve the per-partition
# DMA engines -- 224 GB/s vs 400 GB/s for full-partition alternating).
# Small first groups let the PE start early; small last group keeps the
# post-stream matmul tail short.
GROUPS = [(0, 4), (4, 4), (8, 4), (12, 4), (16, 8), (24, 8), (32, 8), (40, 6)]

TRACE = False

_cache = {}


def chunk_rows(c):
    """Global input-row indices (length ROWS) covered by chunk c, matching
    the merged-group DMA layout: within a group of mg chunks starting at
    chunk c0 (row base r0 = 4*c0), partition (4b+i) holds contiguous rows
    r0 + mg*i + cc for cc in [0, mg)."""
    for c0, mg in GROUPS:
        if c0 <= c < c0 + mg:
            return [4 * c0 + mg * i + (c - c0) for i in range(ROWS)]
    raise ValueError(c)


def _ops():
    """Matmul op schedule: 23 uniform DoubleRow chunk pairs in arrival
    order (chunk 45 -- the bias/tail chunk -- arrives last)."""
    return [("pair", c) for c in range(0, NCHUNK, 2)]


def _build_nc():
    import concourse.bacc as bacc
    import concourse.bass as bass
    import concourse.mybir as mybir
    from concourse.tile import TileContext

    f32 = mybir.dt.float32
    f16 = mybir.dt.float16
    bf16 = mybir.dt.bfloat16
    fp8 = mybir.dt.float8e4
    u32 = mybir.dt.uint32

    nc = bacc.Bacc("TRN2", target_bir_lowering=False, debug=False)

    # fp8 payloads travel as uint32 (4 fp8 lanes per element): the DMA
    # engines are element-rate limited (~14 elem/ns), so 1-byte elements
    # cap a queue-fed engine at ~14 GB/s while 4-byte elements reach the
    # ~22.5 GB/s bus limit.  SBUF tiles are bitcast back to fp8 for the PE.
    x8 = nc.dram_tensor("x8", [NB, IEXT, T // 4], u32, kind="ExternalInput")
    lhsT_full = nc.dram_tensor(
        "lhsT_full", [128, NCHUNK * M // 4], u32, kind="ExternalInput"
    )
    alpha_bc = nc.dram_tensor("alpha_bc", [M, 512], f32, kind="ExternalInput")
    beta_bc = nc.dram_tensor("beta_bc", [M, 512], f16, kind="ExternalInput")
    y = nc.dram_tensor("y", [M, T], f16, kind="ExternalOutput")

    with TileContext(nc) as tc:
        with (
            tc.tile_pool(name="consts", bufs=1) as cpool,
            tc.tile_pool(name="xs", bufs=1) as xpool,
            tc.tile_pool(name="mems", bufs=1) as mpool,
            tc.tile_pool(name="psum", bufs=1, space=bass.MemorySpace.PSUM) as ppool,
        ):
            # Weights for the first 16 chunks ride ahead of x on scalar;
            # the rest follows after the first x group so the slow early
            # DMA-ramp window carries mostly x bytes.
            lwu = cpool.tile([128, NCHUNK, M // 4], u32)
            lhsT_v = lhsT_full[:].rearrange("p (c m) -> p c m", c=NCHUNK)
            nc.scalar.dma_start(out=lwu[:, :16, :], in_=lhsT_v[:, :16, :])
            ab = cpool.tile([M, 512], f32)
            nc.scalar.dma_start(out=ab[:], in_=alpha_bc[:])
            bb = cpool.tile([M, 512], f16)
            nc.scalar.dma_start(out=bb[:], in_=beta_bc[:])

            # full x shard stays resident (92KB/partition in fp8)
            xbu = xpool.tile([128, NCHUNK, T // 4], u32)

            for gi, (c0, mg) in enumerate(GROUPS):
                r0 = 4 * c0
                nrow = ROWS * mg
                src = x8[:, r0 : r0 + nrow, :].rearrange(
                    "b (i cc) t -> b i cc t", i=ROWS, cc=mg
                )
                dst = xbu[:, c0 : c0 + mg, :]
                eng = nc.sync if gi % 2 == 0 else nc.scalar
                eng.dma_start(out=dst, in_=src)
                if gi == 1:
                    nc.scalar.dma_start(
                        out=lwu[:, 16:, :], in_=lhsT_v[:, 16:, :]
                    )

            # fp8 views of the u32-packed tiles for the PE
            lw = lwu[:].bitcast(fp8)  # [128, NCHUNK, M]
            xb = xbu[:].bitcast(fp8)  # [128, NCHUNK, T]

            pt = ppool.tile([M, 2048], f32)

            ops = _ops()
            for k, (kind, c) in enumerate(ops):
                start = k == 0
                stop = k == len(ops) - 1
                off = 0
                for w in TSPLIT:
                    sl = slice(off, off + w)
                    nc.tensor.matmul(
                        pt[:, sl],
                        lw[:, c : c + 2, :],
                        xb[:, c : c + 2, sl],
                        start=start,
                        stop=stop,
                        perf_mode=mybir.MatmulPerfMode.DoubleRow,
                    )
                    off += w

            syn = mpool.tile([M, T], f16)
            mem = mpool.tile([M, T], f16)
            off = 0
            for ti, w in enumerate(TSPLIT):
                sl = slice(off, off + w)
                nc.vector.tensor_tensor_scan(
                    syn[:, sl],
                    ab[:, :w],
                    pt[:, sl],
                    initial=(0.0 if ti == 0 else syn[:, off - 1 : off]),
                    op0=mybir.AluOpType.mult,
                    op1=mybir.AluOpType.add,
                )
                nc.vector.tensor_tensor_scan(
                    mem[:, sl],
                    bb[:, :w],
                    syn[:, sl],
                    initial=(0.0 if ti == 0 else mem[:, off - 1 : off]),
                    op0=mybir.AluOpType.mult,
                    op1=mybir.AluOpType.add,
                )
                eng = nc.sync if ti % 2 == 0 else nc.scalar
                eng.dma_start(out=y[:, sl], in_=mem[:, sl])
                off += w

    nc.compile()
    return nc


def _host_tensors(W, b, alpha, beta):
    import ml_dtypes

    e4 = ml_dtypes.float8_e4m3
    W = np.asarray(W, np.float32)
    bvec = np.asarray(b, np.float32)
    a_cl = np.clip(np.asarray(alpha, np.float32), 0.0, 1.0)
    bt_cl = np.clip(np.asarray(beta, np.float32), 0.0, 1.0)

    # extended weight columns: row 182 multiplies the ones/16 row (bias),
    # row 183 multiplies the zero row
    W_ext = np.zeros((O, IEXT), np.float32)
    W_ext[:, :I] = W
    W_ext[:, I] = bvec / ONES_VAL

    bidx = np.arange(NB)
    lhsT = np.zeros((128, NCHUNK, M), np.float32)
    for c in range(NCHUNK):
        rows = chunk_rows(c)
        for i in range(ROWS):
            for o in range(O):
                lhsT[ROWS * bidx + i, c, 2 * bidx + o] = W_ext[o, rows[i]]

    lhsT_full = np.ascontiguousarray(
        lhsT.reshape(128, NCHUNK * M).astype(e4)
    ).view(np.uint32)
    alpha_bc = np.ascontiguousarray(
        np.broadcast_to(np.tile(a_cl, NB)[:, None], (M, 512))
    ).astype(np.float32)
    beta_bc = np.ascontiguousarray(
        np.broadcast_to(np.tile(bt_cl, NB)[:, None], (M, 512))
    ).astype(np.float16)
    return lhsT_full, alpha_bc, beta_bc


def kernel(inputs, W, b, alpha, beta):
    import ml_dtypes
    from concourse.bass_utils import run_bass_kernel_spmd

    if "fp8" not in _cache:
        _cache["fp8"] = _build_nc()
    nc = _cache["fp8"]

    lhsT_full, alpha_bc, beta_bc = _host_tensors(W, b, alpha, beta)
    # x extended with the ones/16 row (182) and a zero row (183)
    x8f = np.zeros((B, IEXT, T), ml_dtypes.float8_e4m3)
    x8f[:, :I] = np.asarray(inputs, np.float32).astype(ml_dtypes.float8_e4m3)
    x8f[:, I] = ml_dtypes.float8_e4m3(ONES_VAL)
    x8 = np.ascontiguousarray(x8f).view(np.uint32)

    in_maps = []
    for c in range(NCORES):
        in_maps.append(
            {
                "x8": np.ascontiguousarray(x8[c * NB : (c + 1) * NB]),
                "lhsT_full": lhsT_full,
                "alpha_bc": alpha_bc,
                "beta_bc": beta_bc,
            }
        )

    res = run_bass_kernel_spmd(nc, in_maps, core_ids=list(range(NCORES)), trace=TRACE)
    kernel.last_exec_time_ns = res.exec_time_ns
    kernel.last_result = res
    out = np.empty((B, O, T), np.float32)
    for c in range(NCORES):
        out[c * NB : (c + 1) * NB] = (
            res.results[c]["y"].astype(np.float32).reshape(NB, O, T)
        )
    return np.ascontiguousarray(out.transpose(0, 2, 1))


kernel.last_exec_time_ns = None
kernel.last_result = None
